# revision 12
# baseline (speedup 1.0000x reference)
"""DTNNStep (gnn message passing) on 8 Trainium2 NeuronCores.

Strategy (edge-parallel, per the sharding hint):
  * Edges (2M, sorted by membership_i) are sharded across 8 cores at atom
    boundaries: core c owns atoms [12500c, 12500(c+1)) and exactly the edges
    whose destination (membership_i) falls in that range.
  * Within a core, edges are split into 4 substreams by membership_j range
    so the 4 substreams pack the 128-partition segmented scan.
  * Substreams are processed in PAIRS stacked into the 128 PE partitions
    (block-diagonal weights), so each 512-col chunk needs only
    2 dh + 1 ah + 1 fc matmuls per pair (8 matmuls/chunk for all 4
    substreams vs 12 unstacked): PE column-cycles drop 3.0 -> 2.0/edge.
  * Per (pair, chunk): dh matmuls (tile_position 0/64) -> psdh2[128,512];
    ah matmul (block-diag [62,128] weights) -> psah2[128,512]; ACT copies
    psah2 -> SBUF bf16 (DVE cannot read two PSUM operands); DVE multiplies
    psdh2*ah -> prod2; fc matmul (block-diag [128,64]) -> psoh half.
    Gap partitions hold exact zeros (zero weight columns), so no garbage
    propagates. Tanh packs psoh into the [128, c] scan tile; one
    tensor_tensor_scan per c-tile computes masked segmented sums.
  * fin = atom_features - tanh((b_df * a_h) @ W_fc) for the core's own
    atoms (f32).
  * Host: shards/pads inputs (layout only), then reads the scan output at
    (host-known) segment-end columns, adds the 4 substream partials and fin.
"""

import os
import sys

for _p in ("/opt/trn_rl_repo", "/root/.axon_site/_ro/trn_rl_repo"):
    if os.path.isdir(_p) and _p not in sys.path:
        sys.path.append(_p)

import numpy as np
from ml_dtypes import bfloat16
from contextlib import ExitStack

import concourse.bass as bass
import concourse.bacc as bacc
import concourse.mybir as mybir
import concourse.tile as tile
from concourse.bass_utils import run_bass_kernel_spmd

BF16 = mybir.dt.bfloat16
F32 = mybir.dt.float32

MM = 512  # matmul chunk (PSUM bank: 512 f32 per partition)


class Cfg:
    def __init__(self, n_atoms=100000, n_emb=30, n_dist=100, n_hid=60,
                 n_cores=8, n_sub=4, jrange=25000, c=1024, c2=500,
                 jumbo=2048):
        self.n_atoms = n_atoms
        self.n_emb = n_emb
        self.n_dist = n_dist
        self.n_hid = n_hid
        self.n_cores = n_cores
        self.n_sub = n_sub
        self.jrange = jrange
        self.c = c              # scan tile columns
        self.c2 = c2            # fin-phase chunk
        self.apc = n_atoms // n_cores
        self.jumbo = jumbo      # columns per SWDGE bulk DMA (multiple of c)
        assert jumbo % c == 0 and c % MM == 0
        assert self.apc % c2 == 0
        assert jrange * n_sub >= n_atoms
        assert jrange <= 32767


DEFAULT_CFG = Cfg()


def build_program(cfg, cap):
    """Build + compile the (SPMD-identical) Bass program for one core."""
    c = cfg.c
    assert cap % c == 0
    nd1 = cfg.n_dist + 1   # dist rows + ones row
    ne1 = cfg.n_emb + 1    # emb rows + ones row
    H, F = cfg.n_hid, cfg.n_emb

    nc = bacc.Bacc("TRN2", target_bir_lowering=False, debug=False,
                   num_devices=cfg.n_cores, num_swdge_queues=4)

    jb = cfg.jumbo
    nj = cap // jb
    # jumbo-major layouts: one contiguous DRAM block per jumbo load, so each
    # SBUF partition row is a single long descriptor (16KB for dist)
    distJ = nc.dram_tensor("distJ", [nj, nd1, cfg.n_sub, jb], BF16, kind="ExternalInput").ap()
    af2J = nc.dram_tensor("af2J", [nj, 2 * ne1, 2, jb], BF16, kind="ExternalInput").ap()
    maskJ = nc.dram_tensor("maskJ", [nj, 128, jb], mybir.dt.float8e4,
                           kind="ExternalInput").ap()
    a_fT_own = nc.dram_tensor("a_fT_own", [ne1, cfg.apc], BF16, kind="ExternalInput").ap()
    a_f_own = nc.dram_tensor("a_f_own", [cfg.n_emb, cfg.apc], F32, kind="ExternalInput").ap()
    Wdf64 = nc.dram_tensor("Wdf64", [nd1, 64], BF16, kind="ExternalInput").ap()
    Wcf2g = nc.dram_tensor("Wcf2g", [2 * ne1, 128], BF16, kind="ExternalInput").ap()
    Wfc2g = nc.dram_tensor("Wfc2g", [128, 64], BF16, kind="ExternalInput").ap()
    bdf = nc.dram_tensor("bdf", [H, 1], F32, kind="ExternalInput").ap()
    scanout = nc.dram_tensor("scanout", [128, cap], BF16, kind="ExternalOutput").ap()
    fin = nc.dram_tensor("fin", [cfg.n_emb, cfg.apc], F32, kind="ExternalOutput").ap()

    with tile.TileContext(nc) as tc, ExitStack() as ctx:
        wpool = ctx.enter_context(tc.tile_pool(name="weights", bufs=1))
        wdf_sb = wpool.tile([nd1, 64], BF16)
        nc.sync.dma_start(wdf_sb[:], Wdf64[:])
        wcf_sb = wpool.tile([2 * ne1, 128], BF16)
        nc.sync.dma_start(wcf_sb[:], Wcf2g[:])
        wfc_sb = wpool.tile([128, 64], BF16)
        nc.sync.dma_start(wfc_sb[:], Wfc2g[:])
        bdf_sb = wpool.tile([H, 1], F32)
        nc.sync.dma_start(bdf_sb[:], bdf[:])

        # ---------- edge pipeline -------------------------------------------
        # dist + mask go through SWDGE (gpsimd) jumbo DMAs (mask is fp8 in
        # DRAM, cast to bf16 in flight — cast needs SWDGE); af2 + fin ride
        # the SP HWDGE ring and the scanout write-back the Activation HWDGE
        # ring, so the three descriptor generators work in parallel. Loads
        # are dispatched PF jumbos ahead of compute (software pipelining) so
        # in-order engine queues don't throttle prefetch depth.
        tpj = jb // c
        # fin-phase inputs, processed in c2-chunks interleaved with the jumbos
        a_fT_r = a_fT_own
        fin_iters = cfg.apc // cfg.c2
        PF = 3
        with tc.tile_pool(name="ep_d", bufs=4) as dpool, \
             tc.tile_pool(name="ep_a", bufs=4) as apool, \
             tc.tile_pool(name="ep_ah", bufs=4) as ahpool, \
             tc.tile_pool(name="ep_pr", bufs=4) as prpool, \
             tc.tile_pool(name="ep_pk", bufs=2) as ppool, \
             tc.tile_pool(name="ep_mk", bufs=4) as mpool, \
             tc.tile_pool(name="ep_sc", bufs=2) as spool, \
             tc.tile_pool(name="fi_s", bufs=2) as s2, \
             tc.tile_pool(name="ep_ps1", bufs=2, space="PSUM") as ps1, \
             tc.tile_pool(name="ep_ps2", bufs=2, space="PSUM") as ps2, \
             tc.tile_pool(name="ep_ps3", bufs=2, space="PSUM") as ps3, \
             tc.tile_pool(name="fi_ps", bufs=1, space="PSUM") as p2:

            def fin_iter(i):
                q0 = i * cfg.c2
                afo = s2.tile([ne1, cfg.c2], BF16, tag="afo")
                nc.sync.dma_start(afo[:], a_fT_r[:, q0:q0 + cfg.c2])
                aff = s2.tile([cfg.n_emb, cfg.c2], F32, tag="aff")
                nc.sync.dma_start(aff[:], a_f_own[:, q0:q0 + cfg.c2])
                psii = p2.tile([H, cfg.c2], F32, tag="psii")
                nc.tensor.matmul(psii[:], lhsT=wcf_sb[0:ne1, 0:H],
                                 rhs=afo[:], start=True, stop=True)
                pii = s2.tile([H, cfg.c2], BF16, tag="pii")
                nc.scalar.mul(pii[:], psii[:], bdf_sb[:, 0:1])
                psf = p2.tile([F, cfg.c2], F32, tag="psf")
                nc.tensor.matmul(psf[:], lhsT=wfc_sb[0:H, 0:F], rhs=pii[:],
                                 start=True, stop=True)
                th2 = s2.tile([F, cfg.c2], F32, tag="th2")
                nc.scalar.activation(th2[:], psf[:],
                                     mybir.ActivationFunctionType.Tanh)
                fn = s2.tile([F, cfg.c2], F32, tag="fn")
                nc.vector.tensor_tensor(fn[:], aff[:], th2[:],
                                        op=mybir.AluOpType.subtract)
                nc.sync.dma_start(fin[:, q0:q0 + cfg.c2], fn[:])

            loads = {}

            def emit_loads(j):
                dj = dpool.tile([nd1, cfg.n_sub, jb], BF16, tag="dj")
                nc.gpsimd.dma_start(dj[:], distJ[j])
                aj = apool.tile([2 * ne1, 2, jb], BF16, tag="aj")
                nc.sync.dma_start(aj[:], af2J[j])
                mj_ = mpool.tile([128, jb], BF16, tag="mj")
                nc.gpsimd.dma_start(mj_[:], maskJ[j])
                loads[j] = (dj, aj, mj_)

            carry = None
            for j in range(min(PF, nj)):
                emit_loads(j)
            for j in range(nj):
                if j + PF < nj:
                    emit_loads(j + PF)
                dj, aj, mj_ = loads.pop(j)
                stg = spool.tile([128, jb], BF16, tag="stg")
                for tt in range(tpj):
                    c0 = tt * c
                    packed = ppool.tile([128, c], BF16, tag="packed")
                    for q in range(c // MM):
                        n0 = c0 + q * MM
                        psoh = ps3.tile([128, MM], F32, tag="psoh")
                        for p in range(2):
                            psdh2 = ps1.tile([128, MM], F32, tag="psdh")
                            nc.tensor.matmul(psdh2[0:64, :], lhsT=wdf_sb[:],
                                             rhs=dj[:, 2 * p, n0:n0 + MM],
                                             start=True, stop=True,
                                             tile_position=(0, 0))
                            nc.tensor.matmul(psdh2[64:128, :], lhsT=wdf_sb[:],
                                             rhs=dj[:, 2 * p + 1, n0:n0 + MM],
                                             start=True, stop=True,
                                             tile_position=(0, 64))
                            psah2 = ps2.tile([128, MM], F32, tag="psah")
                            nc.tensor.matmul(psah2[:], lhsT=wcf_sb[:],
                                             rhs=aj[:, p, n0:n0 + MM],
                                             start=True, stop=True,
                                             tile_position=(0, 0))
                            ah_sb = ahpool.tile([128, MM], BF16, tag="ah")
                            nc.scalar.copy(ah_sb[:], psah2[:])
                            prod2 = prpool.tile([128, MM], BF16, tag="prod")
                            nc.vector.tensor_tensor(prod2[:], psdh2[:], ah_sb[:],
                                                    op=mybir.AluOpType.mult)
                            nc.tensor.matmul(psoh[64 * p:64 * p + 64, :],
                                             lhsT=wfc_sb[:], rhs=prod2[:],
                                             start=True, stop=True,
                                             tile_position=(0, 64 * p))
                        nc.scalar.activation(packed[:, q * MM:(q + 1) * MM],
                                             psoh[:],
                                             mybir.ActivationFunctionType.Tanh)
                    nc.vector.tensor_tensor_scan(
                        stg[:, c0:c0 + c], data0=mj_[:, c0:c0 + c],
                        data1=packed[:],
                        initial=(0.0 if carry is None else carry),
                        op0=mybir.AluOpType.mult, op1=mybir.AluOpType.add)
                    carry = stg[:, c0 + c - 1:c0 + c]
                nc.scalar.dma_start(scanout[:, j * jb:(j + 1) * jb], stg[:])
                # interleave fin = a_f - tanh((b_df*a_h) @ W_fc) chunks so
                # the tail phase rides the idle gaps of the edge pipeline
                f_lo = fin_iters * j // nj
                f_hi = fin_iters * (j + 1) // nj
                for i in range(f_lo, f_hi):
                    fin_iter(i)

    nc.compile()
    return nc


def host_prep(inputs, cfg):
    """Shard + lay out inputs for the 8 cores. Returns (in_maps, post_data, cap)."""
    af = np.asarray(inputs["atom_features"], dtype=np.float32)
    dist = np.asarray(inputs["distance"], dtype=np.float32)
    mi = np.asarray(inputs["distance_membership_i"]).astype(np.int64)
    mj = np.asarray(inputs["distance_membership_j"]).astype(np.int64)
    W_cf = np.asarray(inputs["W_cf"], dtype=np.float32)
    W_df = np.asarray(inputs["W_df"], dtype=np.float32)
    W_fc = np.asarray(inputs["W_fc"], dtype=np.float32)
    b_cf = np.asarray(inputs["b_cf"], dtype=np.float32)
    b_df = np.asarray(inputs["b_df"], dtype=np.float32)

    n_emb, n_dist, H = cfg.n_emb, cfg.n_dist, cfg.n_hid
    ne1 = n_emb + 1

    Wdf_aug = np.vstack([W_df, b_df[None, :]]).astype(np.float32)   # [101, 60]
    Wcf_aug = np.vstack([W_cf, b_cf[None, :]]).astype(np.float32)   # [31, 60]

    Wdf64 = np.zeros((n_dist + 1, 64), np.float32)
    Wdf64[:, :H] = Wdf_aug
    Wcf2g = np.zeros((2 * ne1, 128), np.float32)
    Wcf2g[0:ne1, 0:H] = Wcf_aug
    Wcf2g[ne1:2 * ne1, 64:64 + H] = Wcf_aug
    Wfc2g = np.zeros((128, 64), np.float32)
    Wfc2g[0:H, 0:n_emb] = W_fc
    Wfc2g[64:64 + H, 32:32 + n_emb] = W_fc
    bdf_col = b_df[:, None].astype(np.float32)

    af_aug = np.concatenate([af, np.ones((cfg.n_atoms, 1), np.float32)], axis=1
                            ).astype(bfloat16)  # [n_atoms, n_emb+1]

    bounds = np.searchsorted(mi, np.arange(0, cfg.n_atoms + 1, cfg.apc))
    core_sels = []
    max_n = 0
    for cid in range(cfg.n_cores):
        e0, e1 = bounds[cid], bounds[cid + 1]
        kk = mj[e0:e1] // cfg.jrange
        sels = [e0 + np.nonzero(kk == k)[0] for k in range(cfg.n_sub)]
        core_sels.append(sels)
        max_n = max(max_n, max(len(s) for s in sels))
    jb = cfg.jumbo
    cap = max(jb, ((max_n + jb - 1) // jb) * jb)

    in_maps = []
    post_data = []
    for cid in range(cfg.n_cores):
        A0 = cid * cfg.apc
        sels = core_sels[cid]
        distT = np.zeros((cfg.n_sub, n_dist + 1, cap), bfloat16)
        af2 = np.zeros((2, 2 * ne1, cap), bfloat16)
        maskx = np.ones((128, cap), np.float32)
        ends_k = []
        for k in range(cfg.n_sub):
            sel = sels[k]
            n = len(sel)
            if n:
                distT[k, :n_dist, :n] = dist[sel].T.astype(bfloat16)
                distT[k, n_dist, :n] = bfloat16(1.0)
                p, half = divmod(k, 2)
                af2[p, half * ne1:(half + 1) * ne1, :n] = af_aug[mj[sel]].T
                ids = mi[sel] - A0
                m = np.ones(cap, np.float32)
                m[0] = 0.0
                m[1:n][ids[1:] != ids[:-1]] = 0.0
                maskx[32 * k:32 * k + n_emb, :] = m[None, :]
                endpos = np.nonzero(np.r_[ids[1:] != ids[:-1], True])[0]
                ends_k.append((endpos.astype(np.int64), ids[endpos].astype(np.int64)))
            else:
                ends_k.append((np.zeros(0, np.int64), np.zeros(0, np.int64)))
        nj = cap // jb
        distJ = np.ascontiguousarray(
            distT.reshape(cfg.n_sub, n_dist + 1, nj, jb).transpose(2, 1, 0, 3))
        af2J = np.ascontiguousarray(
            af2.reshape(2, 2 * ne1, nj, jb).transpose(2, 1, 0, 3))
        maskJ = np.ascontiguousarray(
            maskx.astype(np.dtype("float8_e4m3fn")).reshape(128, nj, jb)
            .transpose(1, 0, 2))
        in_maps.append(dict(
            distJ=distJ,
            af2J=af2J,
            maskJ=maskJ,
            a_fT_own=np.ascontiguousarray(af_aug[A0:A0 + cfg.apc].T),
            a_f_own=np.ascontiguousarray(af[A0:A0 + cfg.apc].T.astype(np.float32)),
            Wdf64=Wdf64.astype(bfloat16), Wcf2g=Wcf2g.astype(bfloat16),
            Wfc2g=Wfc2g.astype(bfloat16), bdf=bdf_col,
        ))
        post_data.append(ends_k)
    return in_maps, post_data, cap


def host_post(results, post_data, cfg):
    out = np.empty((cfg.n_atoms, cfg.n_emb), np.float32)
    for cid in range(cfg.n_cores):
        r = results[cid]
        agg = np.asarray(r["fin"]).astype(np.float32).T.copy()  # [apc, n_emb]
        sc = np.asarray(r["scanout"])  # bf16 [128, cap]
        for k in range(cfg.n_sub):
            endpos, atoms = post_data[cid][k]
            if len(endpos):
                vals = sc[32 * k:32 * k + cfg.n_emb][:, endpos].astype(np.float32)
                np.add.at(agg, atoms, vals.T)
        out[cid * cfg.apc:(cid + 1) * cfg.apc] = agg
    return out


_CACHE = {}


def kernel(**inputs):
    cfg = DEFAULT_CFG
    in_maps, post_data, cap = host_prep(inputs, cfg)
    if cap not in _CACHE:
        _CACHE[cap] = build_program(cfg, cap)
    nc = _CACHE[cap]
    res = run_bass_kernel_spmd(nc, in_maps, core_ids=list(range(cfg.n_cores)))
    return host_post(res.results, post_data, cfg)


# revision 13
# speedup vs baseline: 1.0088x; 1.0088x over previous
"""DTNNStep (gnn message passing) on 8 Trainium2 NeuronCores.

Strategy (edge-parallel, per the sharding hint):
  * Edges (2M, sorted by membership_i) are sharded across 8 cores at atom
    boundaries: core c owns atoms [12500c, 12500(c+1)) and exactly the edges
    whose destination (membership_i) falls in that range.
  * Within a core, edges are split into 4 substreams by membership_j range
    so the 4 substreams pack the 128-partition segmented scan.
  * Substreams are processed in PAIRS stacked into the 128 PE partitions
    (block-diagonal weights), so each 512-col chunk needs only
    2 dh + 1 ah + 1 fc matmuls per pair (8 matmuls/chunk for all 4
    substreams vs 12 unstacked): PE column-cycles drop 3.0 -> 2.0/edge.
  * Per (pair, chunk): dh matmuls (tile_position 0/64) -> psdh2[128,512];
    ah matmul (block-diag [62,128] weights) -> psah2[128,512]; ACT copies
    psah2 -> SBUF bf16 (DVE cannot read two PSUM operands); DVE multiplies
    psdh2*ah -> prod2; fc matmul (block-diag [128,64]) -> psoh half.
    Gap partitions hold exact zeros (zero weight columns), so no garbage
    propagates. Tanh packs psoh into the [128, c] scan tile; one
    tensor_tensor_scan per c-tile computes masked segmented sums.
  * fin = atom_features - tanh((b_df * a_h) @ W_fc) for the core's own
    atoms (f32).
  * Host: shards/pads inputs (layout only), then reads the scan output at
    (host-known) segment-end columns, adds the 4 substream partials and fin.
"""

import os
import sys

for _p in ("/opt/trn_rl_repo", "/root/.axon_site/_ro/trn_rl_repo"):
    if os.path.isdir(_p) and _p not in sys.path:
        sys.path.append(_p)

import numpy as np
from ml_dtypes import bfloat16
from contextlib import ExitStack

import concourse.bass as bass
import concourse.bacc as bacc
import concourse.mybir as mybir
import concourse.tile as tile
from concourse.bass_utils import run_bass_kernel_spmd

BF16 = mybir.dt.bfloat16
F32 = mybir.dt.float32

MM = 512  # matmul chunk (PSUM bank: 512 f32 per partition)


class Cfg:
    def __init__(self, n_atoms=100000, n_emb=30, n_dist=100, n_hid=60,
                 n_cores=8, n_sub=4, jrange=25000, c=1024, c2=500,
                 jumbo=2048):
        self.n_atoms = n_atoms
        self.n_emb = n_emb
        self.n_dist = n_dist
        self.n_hid = n_hid
        self.n_cores = n_cores
        self.n_sub = n_sub
        self.jrange = jrange
        self.c = c              # scan tile columns
        self.c2 = c2            # fin-phase chunk
        self.apc = n_atoms // n_cores
        self.jumbo = jumbo      # columns per SWDGE bulk DMA (multiple of c)
        assert jumbo % c == 0 and c % MM == 0
        assert self.apc % c2 == 0
        assert jrange * n_sub >= n_atoms
        assert jrange <= 32767


DEFAULT_CFG = Cfg()


def build_program(cfg, cap):
    """Build + compile the (SPMD-identical) Bass program for one core."""
    c = cfg.c
    assert cap % c == 0
    nd1 = cfg.n_dist + 1   # dist rows + ones row
    ne1 = cfg.n_emb + 1    # emb rows + ones row
    H, F = cfg.n_hid, cfg.n_emb

    nc = bacc.Bacc("TRN2", target_bir_lowering=False, debug=False,
                   num_devices=cfg.n_cores, num_swdge_queues=4)

    jb = cfg.jumbo
    nj = cap // jb
    # jumbo-major layouts: one contiguous DRAM block per jumbo load, so each
    # SBUF partition row is a single long descriptor (16KB for dist)
    distJ = nc.dram_tensor("distJ", [nj, nd1, cfg.n_sub, jb], BF16, kind="ExternalInput").ap()
    af2J = nc.dram_tensor("af2J", [nj, 2 * ne1, 2, jb], BF16, kind="ExternalInput").ap()
    maskJ = nc.dram_tensor("maskJ", [nj, 128, jb], mybir.dt.float8e4,
                           kind="ExternalInput").ap()
    a_fT_own = nc.dram_tensor("a_fT_own", [ne1, cfg.apc], BF16, kind="ExternalInput").ap()
    a_f_own = nc.dram_tensor("a_f_own", [cfg.n_emb, cfg.apc], F32, kind="ExternalInput").ap()
    Wdf64 = nc.dram_tensor("Wdf64", [nd1, 64], BF16, kind="ExternalInput").ap()
    Wcf2g = nc.dram_tensor("Wcf2g", [2 * ne1, 128], BF16, kind="ExternalInput").ap()
    Wfc2g = nc.dram_tensor("Wfc2g", [128, 64], BF16, kind="ExternalInput").ap()
    bdf = nc.dram_tensor("bdf", [H, 1], F32, kind="ExternalInput").ap()
    scanout = nc.dram_tensor("scanout", [128, cap], BF16, kind="ExternalOutput").ap()
    fin = nc.dram_tensor("fin", [cfg.n_emb, cfg.apc], F32, kind="ExternalOutput").ap()

    with tile.TileContext(nc) as tc, ExitStack() as ctx:
        wpool = ctx.enter_context(tc.tile_pool(name="weights", bufs=1))
        wdf_sb = wpool.tile([nd1, 64], BF16)
        nc.sync.dma_start(wdf_sb[:], Wdf64[:])
        wcf_sb = wpool.tile([2 * ne1, 128], BF16)
        nc.sync.dma_start(wcf_sb[:], Wcf2g[:])
        wfc_sb = wpool.tile([128, 64], BF16)
        nc.sync.dma_start(wfc_sb[:], Wfc2g[:])
        bdf_sb = wpool.tile([H, 1], F32)
        nc.sync.dma_start(bdf_sb[:], bdf[:])

        # ---------- edge pipeline -------------------------------------------
        # dist + mask go through SWDGE (gpsimd) jumbo DMAs (mask is fp8 in
        # DRAM, cast to bf16 in flight — cast needs SWDGE); af2 + fin ride
        # the SP HWDGE ring and the scanout write-back the Activation HWDGE
        # ring, so the three descriptor generators work in parallel. Loads
        # are dispatched PF jumbos ahead of compute (software pipelining) so
        # in-order engine queues don't throttle prefetch depth.
        tpj = jb // c
        # fin-phase inputs, processed in c2-chunks interleaved with the jumbos
        a_fT_r = a_fT_own
        fin_iters = cfg.apc // cfg.c2
        PF = 3
        with tc.tile_pool(name="ep_d", bufs=4) as dpool, \
             tc.tile_pool(name="ep_a", bufs=4) as apool, \
             tc.tile_pool(name="ep_ah", bufs=4) as ahpool, \
             tc.tile_pool(name="ep_pr", bufs=4) as prpool, \
             tc.tile_pool(name="ep_pk", bufs=2) as ppool, \
             tc.tile_pool(name="ep_mk", bufs=4) as mpool, \
             tc.tile_pool(name="ep_sc", bufs=2) as spool, \
             tc.tile_pool(name="fi_s", bufs=2) as s2, \
             tc.tile_pool(name="ep_ps1", bufs=2, space="PSUM") as ps1, \
             tc.tile_pool(name="ep_ps2", bufs=2, space="PSUM") as ps2, \
             tc.tile_pool(name="ep_ps3", bufs=2, space="PSUM") as ps3, \
             tc.tile_pool(name="fi_ps", bufs=1, space="PSUM") as p2:

            def fin_iter(i):
                q0 = i * cfg.c2
                afo = s2.tile([ne1, cfg.c2], BF16, tag="afo")
                nc.sync.dma_start(afo[:], a_fT_r[:, q0:q0 + cfg.c2])
                aff = s2.tile([cfg.n_emb, cfg.c2], F32, tag="aff")
                nc.sync.dma_start(aff[:], a_f_own[:, q0:q0 + cfg.c2])
                psii = p2.tile([H, cfg.c2], F32, tag="psii")
                nc.tensor.matmul(psii[:], lhsT=wcf_sb[0:ne1, 0:H],
                                 rhs=afo[:], start=True, stop=True)
                pii = s2.tile([H, cfg.c2], BF16, tag="pii")
                nc.scalar.mul(pii[:], psii[:], bdf_sb[:, 0:1])
                psf = p2.tile([F, cfg.c2], F32, tag="psf")
                nc.tensor.matmul(psf[:], lhsT=wfc_sb[0:H, 0:F], rhs=pii[:],
                                 start=True, stop=True)
                th2 = s2.tile([F, cfg.c2], F32, tag="th2")
                nc.scalar.activation(th2[:], psf[:],
                                     mybir.ActivationFunctionType.Tanh)
                fn = s2.tile([F, cfg.c2], F32, tag="fn")
                nc.vector.tensor_tensor(fn[:], aff[:], th2[:],
                                        op=mybir.AluOpType.subtract)
                nc.sync.dma_start(fin[:, q0:q0 + cfg.c2], fn[:])

            loads = {}

            def emit_loads(j):
                dj = dpool.tile([nd1, cfg.n_sub, jb], BF16, tag="dj")
                nc.gpsimd.dma_start(dj[:], distJ[j])
                aj = apool.tile([2 * ne1, 2, jb], BF16, tag="aj")
                nc.gpsimd.dma_start(aj[:], af2J[j])
                mj_ = mpool.tile([128, jb], BF16, tag="mj")
                nc.gpsimd.dma_start(mj_[:], maskJ[j])
                loads[j] = (dj, aj, mj_)

            carry = None
            for j in range(min(PF, nj)):
                emit_loads(j)
            for j in range(nj):
                if j + PF < nj:
                    emit_loads(j + PF)
                dj, aj, mj_ = loads.pop(j)
                stg = spool.tile([128, jb], BF16, tag="stg")
                for tt in range(tpj):
                    c0 = tt * c
                    packed = ppool.tile([128, c], BF16, tag="packed")
                    for q in range(c // MM):
                        n0 = c0 + q * MM
                        psoh = ps3.tile([128, MM], F32, tag="psoh")
                        for p in range(2):
                            psdh2 = ps1.tile([128, MM], F32, tag="psdh")
                            nc.tensor.matmul(psdh2[0:64, :], lhsT=wdf_sb[:],
                                             rhs=dj[:, 2 * p, n0:n0 + MM],
                                             start=True, stop=True,
                                             tile_position=(0, 0))
                            nc.tensor.matmul(psdh2[64:128, :], lhsT=wdf_sb[:],
                                             rhs=dj[:, 2 * p + 1, n0:n0 + MM],
                                             start=True, stop=True,
                                             tile_position=(0, 64))
                            psah2 = ps2.tile([128, MM], F32, tag="psah")
                            nc.tensor.matmul(psah2[:], lhsT=wcf_sb[:],
                                             rhs=aj[:, p, n0:n0 + MM],
                                             start=True, stop=True,
                                             tile_position=(0, 0))
                            ah_sb = ahpool.tile([128, MM], BF16, tag="ah")
                            nc.scalar.copy(ah_sb[:], psah2[:])
                            prod2 = prpool.tile([128, MM], BF16, tag="prod")
                            nc.vector.tensor_tensor(prod2[:], psdh2[:], ah_sb[:],
                                                    op=mybir.AluOpType.mult)
                            nc.tensor.matmul(psoh[64 * p:64 * p + 64, :],
                                             lhsT=wfc_sb[:], rhs=prod2[:],
                                             start=True, stop=True,
                                             tile_position=(0, 64 * p))
                        nc.scalar.activation(packed[:, q * MM:(q + 1) * MM],
                                             psoh[:],
                                             mybir.ActivationFunctionType.Tanh)
                    nc.vector.tensor_tensor_scan(
                        stg[:, c0:c0 + c], data0=mj_[:, c0:c0 + c],
                        data1=packed[:],
                        initial=(0.0 if carry is None else carry),
                        op0=mybir.AluOpType.mult, op1=mybir.AluOpType.add)
                    carry = stg[:, c0 + c - 1:c0 + c]
                nc.scalar.dma_start(scanout[:, j * jb:(j + 1) * jb], stg[:])
                # interleave fin = a_f - tanh((b_df*a_h) @ W_fc) chunks so
                # the tail phase rides the idle gaps of the edge pipeline
                f_lo = fin_iters * j // nj
                f_hi = fin_iters * (j + 1) // nj
                for i in range(f_lo, f_hi):
                    fin_iter(i)

    nc.compile()
    return nc


def host_prep(inputs, cfg):
    """Shard + lay out inputs for the 8 cores. Returns (in_maps, post_data, cap)."""
    af = np.asarray(inputs["atom_features"], dtype=np.float32)
    dist = np.asarray(inputs["distance"], dtype=np.float32)
    mi = np.asarray(inputs["distance_membership_i"]).astype(np.int64)
    mj = np.asarray(inputs["distance_membership_j"]).astype(np.int64)
    W_cf = np.asarray(inputs["W_cf"], dtype=np.float32)
    W_df = np.asarray(inputs["W_df"], dtype=np.float32)
    W_fc = np.asarray(inputs["W_fc"], dtype=np.float32)
    b_cf = np.asarray(inputs["b_cf"], dtype=np.float32)
    b_df = np.asarray(inputs["b_df"], dtype=np.float32)

    n_emb, n_dist, H = cfg.n_emb, cfg.n_dist, cfg.n_hid
    ne1 = n_emb + 1

    Wdf_aug = np.vstack([W_df, b_df[None, :]]).astype(np.float32)   # [101, 60]
    Wcf_aug = np.vstack([W_cf, b_cf[None, :]]).astype(np.float32)   # [31, 60]

    Wdf64 = np.zeros((n_dist + 1, 64), np.float32)
    Wdf64[:, :H] = Wdf_aug
    Wcf2g = np.zeros((2 * ne1, 128), np.float32)
    Wcf2g[0:ne1, 0:H] = Wcf_aug
    Wcf2g[ne1:2 * ne1, 64:64 + H] = Wcf_aug
    Wfc2g = np.zeros((128, 64), np.float32)
    Wfc2g[0:H, 0:n_emb] = W_fc
    Wfc2g[64:64 + H, 32:32 + n_emb] = W_fc
    bdf_col = b_df[:, None].astype(np.float32)

    af_aug = np.concatenate([af, np.ones((cfg.n_atoms, 1), np.float32)], axis=1
                            ).astype(bfloat16)  # [n_atoms, n_emb+1]

    bounds = np.searchsorted(mi, np.arange(0, cfg.n_atoms + 1, cfg.apc))
    core_sels = []
    max_n = 0
    for cid in range(cfg.n_cores):
        e0, e1 = bounds[cid], bounds[cid + 1]
        kk = mj[e0:e1] // cfg.jrange
        sels = [e0 + np.nonzero(kk == k)[0] for k in range(cfg.n_sub)]
        core_sels.append(sels)
        max_n = max(max_n, max(len(s) for s in sels))
    jb = cfg.jumbo
    cap = max(jb, ((max_n + jb - 1) // jb) * jb)

    in_maps = []
    post_data = []
    for cid in range(cfg.n_cores):
        A0 = cid * cfg.apc
        sels = core_sels[cid]
        distT = np.zeros((cfg.n_sub, n_dist + 1, cap), bfloat16)
        af2 = np.zeros((2, 2 * ne1, cap), bfloat16)
        maskx = np.ones((128, cap), np.float32)
        ends_k = []
        for k in range(cfg.n_sub):
            sel = sels[k]
            n = len(sel)
            if n:
                distT[k, :n_dist, :n] = dist[sel].T.astype(bfloat16)
                distT[k, n_dist, :n] = bfloat16(1.0)
                p, half = divmod(k, 2)
                af2[p, half * ne1:(half + 1) * ne1, :n] = af_aug[mj[sel]].T
                ids = mi[sel] - A0
                m = np.ones(cap, np.float32)
                m[0] = 0.0
                m[1:n][ids[1:] != ids[:-1]] = 0.0
                maskx[32 * k:32 * k + n_emb, :] = m[None, :]
                endpos = np.nonzero(np.r_[ids[1:] != ids[:-1], True])[0]
                ends_k.append((endpos.astype(np.int64), ids[endpos].astype(np.int64)))
            else:
                ends_k.append((np.zeros(0, np.int64), np.zeros(0, np.int64)))
        nj = cap // jb
        distJ = np.ascontiguousarray(
            distT.reshape(cfg.n_sub, n_dist + 1, nj, jb).transpose(2, 1, 0, 3))
        af2J = np.ascontiguousarray(
            af2.reshape(2, 2 * ne1, nj, jb).transpose(2, 1, 0, 3))
        maskJ = np.ascontiguousarray(
            maskx.astype(np.dtype("float8_e4m3fn")).reshape(128, nj, jb)
            .transpose(1, 0, 2))
        in_maps.append(dict(
            distJ=distJ,
            af2J=af2J,
            maskJ=maskJ,
            a_fT_own=np.ascontiguousarray(af_aug[A0:A0 + cfg.apc].T),
            a_f_own=np.ascontiguousarray(af[A0:A0 + cfg.apc].T.astype(np.float32)),
            Wdf64=Wdf64.astype(bfloat16), Wcf2g=Wcf2g.astype(bfloat16),
            Wfc2g=Wfc2g.astype(bfloat16), bdf=bdf_col,
        ))
        post_data.append(ends_k)
    return in_maps, post_data, cap


def host_post(results, post_data, cfg):
    out = np.empty((cfg.n_atoms, cfg.n_emb), np.float32)
    for cid in range(cfg.n_cores):
        r = results[cid]
        agg = np.asarray(r["fin"]).astype(np.float32).T.copy()  # [apc, n_emb]
        sc = np.asarray(r["scanout"])  # bf16 [128, cap]
        for k in range(cfg.n_sub):
            endpos, atoms = post_data[cid][k]
            if len(endpos):
                vals = sc[32 * k:32 * k + cfg.n_emb][:, endpos].astype(np.float32)
                np.add.at(agg, atoms, vals.T)
        out[cid * cfg.apc:(cid + 1) * cfg.apc] = agg
    return out


_CACHE = {}


def kernel(**inputs):
    cfg = DEFAULT_CFG
    in_maps, post_data, cap = host_prep(inputs, cfg)
    if cap not in _CACHE:
        _CACHE[cap] = build_program(cfg, cap)
    nc = _CACHE[cap]
    res = run_bass_kernel_spmd(nc, in_maps, core_ids=list(range(cfg.n_cores)))
    return host_post(res.results, post_data, cfg)


# revision 14
# speedup vs baseline: 1.0899x; 1.0803x over previous
"""DTNNStep (gnn message passing) on 8 Trainium2 NeuronCores.

Strategy (edge-parallel, per the sharding hint):
  * Edges (2M, sorted by membership_i) are sharded across 8 cores at atom
    boundaries: core c owns atoms [12500c, 12500(c+1)) and exactly the edges
    whose destination (membership_i) falls in that range.
  * Within a core, edges are split into 4 substreams by membership_j range
    so the 4 substreams pack the 128-partition segmented scan.
  * Substreams are processed in PAIRS stacked into the 128 PE partitions
    (block-diagonal weights), so each 512-col chunk needs only
    2 dh + 1 ah + 1 fc matmuls per pair (8 matmuls/chunk for all 4
    substreams vs 12 unstacked): PE column-cycles drop 3.0 -> 2.0/edge.
  * Per (pair, chunk): dh matmuls (tile_position 0/64) -> psdh2[128,512];
    ah matmul (block-diag [62,128] weights) -> psah2[128,512]; ACT copies
    psah2 -> SBUF bf16 (DVE cannot read two PSUM operands); DVE multiplies
    psdh2*ah -> prod2; fc matmul (block-diag [128,64]) -> psoh half.
    Gap partitions hold exact zeros (zero weight columns), so no garbage
    propagates. Tanh packs psoh into the [128, c] scan tile; one
    tensor_tensor_scan per c-tile computes masked segmented sums.
  * fin = atom_features - tanh((b_df * a_h) @ W_fc) for the core's own
    atoms (f32).
  * Host: shards/pads inputs (layout only), then reads the scan output at
    (host-known) segment-end columns, adds the 4 substream partials and fin.
"""

import os
import sys

for _p in ("/opt/trn_rl_repo", "/root/.axon_site/_ro/trn_rl_repo"):
    if os.path.isdir(_p) and _p not in sys.path:
        sys.path.append(_p)

import numpy as np
from ml_dtypes import bfloat16
from contextlib import ExitStack

import concourse.bass as bass
import concourse.bacc as bacc
import concourse.mybir as mybir
import concourse.tile as tile
from concourse.bass_utils import run_bass_kernel_spmd

BF16 = mybir.dt.bfloat16
F32 = mybir.dt.float32

MM = 512  # matmul chunk (PSUM bank: 512 f32 per partition)


class Cfg:
    def __init__(self, n_atoms=100000, n_emb=30, n_dist=100, n_hid=60,
                 n_cores=8, n_sub=4, jrange=25000, c=1024, c2=500,
                 jumbo=2048):
        self.n_atoms = n_atoms
        self.n_emb = n_emb
        self.n_dist = n_dist
        self.n_hid = n_hid
        self.n_cores = n_cores
        self.n_sub = n_sub
        self.jrange = jrange
        self.c = c              # scan tile columns
        self.c2 = c2            # fin-phase chunk
        self.apc = n_atoms // n_cores
        self.jumbo = jumbo      # columns per SWDGE bulk DMA (multiple of c)
        assert jumbo % c == 0 and c % MM == 0
        assert self.apc % c2 == 0
        assert jrange * n_sub >= n_atoms
        assert jrange <= 32767


DEFAULT_CFG = Cfg()


def build_program(cfg, cap):
    """Build + compile the (SPMD-identical) Bass program for one core."""
    c = cfg.c
    assert cap % c == 0
    nd1 = cfg.n_dist + 1   # dist rows + ones row
    ne1 = cfg.n_emb + 1    # emb rows + ones row
    H, F = cfg.n_hid, cfg.n_emb

    nc = bacc.Bacc("TRN2", target_bir_lowering=False, debug=False,
                   num_devices=cfg.n_cores, num_swdge_queues=4)

    jb = cfg.jumbo
    nj = cap // jb
    # jumbo-major layouts: one contiguous DRAM block per jumbo load, so each
    # SBUF partition row is a single long descriptor (16KB for dist)
    distJ = nc.dram_tensor("distJ", [nj, nd1, cfg.n_sub, jb], BF16, kind="ExternalInput").ap()
    af2J = nc.dram_tensor("af2J", [nj, 2 * ne1, 2, jb], BF16, kind="ExternalInput").ap()
    maskJ = nc.dram_tensor("maskJ", [nj, 128, jb], mybir.dt.float8e4,
                           kind="ExternalInput").ap()
    a_fT_own = nc.dram_tensor("a_fT_own", [ne1, cfg.apc], BF16, kind="ExternalInput").ap()
    a_f_own = nc.dram_tensor("a_f_own", [cfg.n_emb, cfg.apc], F32, kind="ExternalInput").ap()
    Wdf64 = nc.dram_tensor("Wdf64", [nd1, 64], BF16, kind="ExternalInput").ap()
    Wcf2g = nc.dram_tensor("Wcf2g", [2 * ne1, 128], BF16, kind="ExternalInput").ap()
    Wfc2g = nc.dram_tensor("Wfc2g", [128, 64], BF16, kind="ExternalInput").ap()
    bdf = nc.dram_tensor("bdf", [H, 1], F32, kind="ExternalInput").ap()
    scanout = nc.dram_tensor("scanout", [128, cap], BF16, kind="ExternalOutput").ap()
    fin = nc.dram_tensor("fin", [cfg.n_emb, cfg.apc], F32, kind="ExternalOutput").ap()

    with tile.TileContext(nc) as tc, ExitStack() as ctx:
        wpool = ctx.enter_context(tc.tile_pool(name="weights", bufs=1))
        wdf_sb = wpool.tile([nd1, 64], BF16)
        nc.sync.dma_start(wdf_sb[:], Wdf64[:])
        wcf_sb = wpool.tile([2 * ne1, 128], BF16)
        nc.sync.dma_start(wcf_sb[:], Wcf2g[:])
        wfc_sb = wpool.tile([128, 64], BF16)
        nc.sync.dma_start(wfc_sb[:], Wfc2g[:])
        bdf_sb = wpool.tile([H, 1], F32)
        nc.sync.dma_start(bdf_sb[:], bdf[:])

        # ---------- edge pipeline -------------------------------------------
        # dist + mask go through SWDGE (gpsimd) jumbo DMAs (mask is fp8 in
        # DRAM, cast to bf16 in flight — cast needs SWDGE); af2 + fin ride
        # the SP HWDGE ring and the scanout write-back the Activation HWDGE
        # ring, so the three descriptor generators work in parallel. Loads
        # are dispatched PF jumbos ahead of compute (software pipelining) so
        # in-order engine queues don't throttle prefetch depth.
        tpj = jb // c
        # fin-phase inputs, processed in c2-chunks interleaved with the jumbos
        a_fT_r = a_fT_own
        fin_iters = cfg.apc // cfg.c2
        PF = 3
        with tc.tile_pool(name="ep_d", bufs=4) as dpool, \
             tc.tile_pool(name="ep_a", bufs=4) as apool, \
             tc.tile_pool(name="ep_ah", bufs=4) as ahpool, \
             tc.tile_pool(name="ep_pr", bufs=4) as prpool, \
             tc.tile_pool(name="ep_pk", bufs=2) as ppool, \
             tc.tile_pool(name="ep_mk", bufs=4) as mpool, \
             tc.tile_pool(name="ep_sc", bufs=2) as spool, \
             tc.tile_pool(name="fi_s", bufs=2) as s2, \
             tc.tile_pool(name="ep_ps1", bufs=2, space="PSUM") as ps1, \
             tc.tile_pool(name="ep_ps2", bufs=2, space="PSUM") as ps2, \
             tc.tile_pool(name="ep_ps3", bufs=2, space="PSUM") as ps3, \
             tc.tile_pool(name="fi_ps", bufs=1, space="PSUM") as p2:

            def fin_iter(i):
                q0 = i * cfg.c2
                afo = s2.tile([ne1, cfg.c2], BF16, tag="afo")
                nc.sync.dma_start(afo[:], a_fT_r[:, q0:q0 + cfg.c2])
                aff = s2.tile([cfg.n_emb, cfg.c2], F32, tag="aff")
                nc.sync.dma_start(aff[:], a_f_own[:, q0:q0 + cfg.c2])
                psii = p2.tile([H, cfg.c2], F32, tag="psii")
                nc.tensor.matmul(psii[:], lhsT=wcf_sb[0:ne1, 0:H],
                                 rhs=afo[:], start=True, stop=True)
                pii = s2.tile([H, cfg.c2], BF16, tag="pii")
                nc.scalar.mul(pii[:], psii[:], bdf_sb[:, 0:1])
                psf = p2.tile([F, cfg.c2], F32, tag="psf")
                nc.tensor.matmul(psf[:], lhsT=wfc_sb[0:H, 0:F], rhs=pii[:],
                                 start=True, stop=True)
                th2 = s2.tile([F, cfg.c2], F32, tag="th2")
                nc.scalar.activation(th2[:], psf[:],
                                     mybir.ActivationFunctionType.Tanh)
                fn = s2.tile([F, cfg.c2], F32, tag="fn")
                nc.vector.tensor_tensor(fn[:], aff[:], th2[:],
                                        op=mybir.AluOpType.subtract)
                nc.sync.dma_start(fin[:, q0:q0 + cfg.c2], fn[:])

            loads = {}

            # each SWDGE instruction drains on ~one SDMA engine in this
            # runtime, so split every load into row-bands: concurrency from
            # many instructions, efficiency from long (16KB) per-partition
            # descriptors
            def emit_loads(j):
                dj = dpool.tile([nd1, cfg.n_sub, jb], BF16, tag="dj")
                for r0 in range(0, nd1, 17):
                    r1 = min(r0 + 17, nd1)
                    nc.gpsimd.dma_start(dj[r0:r1], distJ[j, r0:r1])
                aj = apool.tile([2 * ne1, 2, jb], BF16, tag="aj")
                for r0 in range(0, 2 * ne1, 16):
                    r1 = min(r0 + 16, 2 * ne1)
                    nc.gpsimd.dma_start(aj[r0:r1], af2J[j, r0:r1])
                mj_ = mpool.tile([128, jb], mybir.dt.float8e4, tag="mj")
                nc.sync.dma_start(mj_[:], maskJ[j])
                loads[j] = (dj, aj, mj_)

            carry = None
            for j in range(min(PF, nj)):
                emit_loads(j)
            for j in range(nj):
                if j + PF < nj:
                    emit_loads(j + PF)
                dj, aj, mj_ = loads.pop(j)
                stg = spool.tile([128, jb], BF16, tag="stg")
                for tt in range(tpj):
                    c0 = tt * c
                    packed = ppool.tile([128, c], BF16, tag="packed")
                    for q in range(c // MM):
                        n0 = c0 + q * MM
                        psoh = ps3.tile([128, MM], F32, tag="psoh")
                        for p in range(2):
                            psdh2 = ps1.tile([128, MM], F32, tag="psdh")
                            nc.tensor.matmul(psdh2[0:64, :], lhsT=wdf_sb[:],
                                             rhs=dj[:, 2 * p, n0:n0 + MM],
                                             start=True, stop=True,
                                             tile_position=(0, 0))
                            nc.tensor.matmul(psdh2[64:128, :], lhsT=wdf_sb[:],
                                             rhs=dj[:, 2 * p + 1, n0:n0 + MM],
                                             start=True, stop=True,
                                             tile_position=(0, 64))
                            psah2 = ps2.tile([128, MM], F32, tag="psah")
                            nc.tensor.matmul(psah2[:], lhsT=wcf_sb[:],
                                             rhs=aj[:, p, n0:n0 + MM],
                                             start=True, stop=True,
                                             tile_position=(0, 0))
                            ah_sb = ahpool.tile([128, MM], BF16, tag="ah")
                            nc.scalar.copy(ah_sb[:], psah2[:])
                            prod2 = prpool.tile([128, MM], BF16, tag="prod")
                            nc.vector.tensor_tensor(prod2[:], psdh2[:], ah_sb[:],
                                                    op=mybir.AluOpType.mult)
                            nc.tensor.matmul(psoh[64 * p:64 * p + 64, :],
                                             lhsT=wfc_sb[:], rhs=prod2[:],
                                             start=True, stop=True,
                                             tile_position=(0, 64 * p))
                        nc.scalar.activation(packed[:, q * MM:(q + 1) * MM],
                                             psoh[:],
                                             mybir.ActivationFunctionType.Tanh)
                    nc.vector.tensor_tensor_scan(
                        stg[:, c0:c0 + c], data0=mj_[:, c0:c0 + c],
                        data1=packed[:],
                        initial=(0.0 if carry is None else carry),
                        op0=mybir.AluOpType.mult, op1=mybir.AluOpType.add)
                    carry = stg[:, c0 + c - 1:c0 + c]
                nc.scalar.dma_start(scanout[:, j * jb:(j + 1) * jb], stg[:])
                # interleave fin = a_f - tanh((b_df*a_h) @ W_fc) chunks so
                # the tail phase rides the idle gaps of the edge pipeline
                f_lo = fin_iters * j // nj
                f_hi = fin_iters * (j + 1) // nj
                for i in range(f_lo, f_hi):
                    fin_iter(i)

    nc.compile()
    return nc


def host_prep(inputs, cfg):
    """Shard + lay out inputs for the 8 cores. Returns (in_maps, post_data, cap)."""
    af = np.asarray(inputs["atom_features"], dtype=np.float32)
    dist = np.asarray(inputs["distance"], dtype=np.float32)
    mi = np.asarray(inputs["distance_membership_i"]).astype(np.int64)
    mj = np.asarray(inputs["distance_membership_j"]).astype(np.int64)
    W_cf = np.asarray(inputs["W_cf"], dtype=np.float32)
    W_df = np.asarray(inputs["W_df"], dtype=np.float32)
    W_fc = np.asarray(inputs["W_fc"], dtype=np.float32)
    b_cf = np.asarray(inputs["b_cf"], dtype=np.float32)
    b_df = np.asarray(inputs["b_df"], dtype=np.float32)

    n_emb, n_dist, H = cfg.n_emb, cfg.n_dist, cfg.n_hid
    ne1 = n_emb + 1

    Wdf_aug = np.vstack([W_df, b_df[None, :]]).astype(np.float32)   # [101, 60]
    Wcf_aug = np.vstack([W_cf, b_cf[None, :]]).astype(np.float32)   # [31, 60]

    Wdf64 = np.zeros((n_dist + 1, 64), np.float32)
    Wdf64[:, :H] = Wdf_aug
    Wcf2g = np.zeros((2 * ne1, 128), np.float32)
    Wcf2g[0:ne1, 0:H] = Wcf_aug
    Wcf2g[ne1:2 * ne1, 64:64 + H] = Wcf_aug
    Wfc2g = np.zeros((128, 64), np.float32)
    Wfc2g[0:H, 0:n_emb] = W_fc
    Wfc2g[64:64 + H, 32:32 + n_emb] = W_fc
    bdf_col = b_df[:, None].astype(np.float32)

    af_aug = np.concatenate([af, np.ones((cfg.n_atoms, 1), np.float32)], axis=1
                            ).astype(bfloat16)  # [n_atoms, n_emb+1]

    bounds = np.searchsorted(mi, np.arange(0, cfg.n_atoms + 1, cfg.apc))
    core_sels = []
    max_n = 0
    for cid in range(cfg.n_cores):
        e0, e1 = bounds[cid], bounds[cid + 1]
        kk = mj[e0:e1] // cfg.jrange
        sels = [e0 + np.nonzero(kk == k)[0] for k in range(cfg.n_sub)]
        core_sels.append(sels)
        max_n = max(max_n, max(len(s) for s in sels))
    jb = cfg.jumbo
    cap = max(jb, ((max_n + jb - 1) // jb) * jb)

    in_maps = []
    post_data = []
    for cid in range(cfg.n_cores):
        A0 = cid * cfg.apc
        sels = core_sels[cid]
        distT = np.zeros((cfg.n_sub, n_dist + 1, cap), bfloat16)
        af2 = np.zeros((2, 2 * ne1, cap), bfloat16)
        maskx = np.ones((128, cap), np.float32)
        ends_k = []
        for k in range(cfg.n_sub):
            sel = sels[k]
            n = len(sel)
            if n:
                distT[k, :n_dist, :n] = dist[sel].T.astype(bfloat16)
                distT[k, n_dist, :n] = bfloat16(1.0)
                p, half = divmod(k, 2)
                af2[p, half * ne1:(half + 1) * ne1, :n] = af_aug[mj[sel]].T
                ids = mi[sel] - A0
                m = np.ones(cap, np.float32)
                m[0] = 0.0
                m[1:n][ids[1:] != ids[:-1]] = 0.0
                maskx[32 * k:32 * k + n_emb, :] = m[None, :]
                endpos = np.nonzero(np.r_[ids[1:] != ids[:-1], True])[0]
                ends_k.append((endpos.astype(np.int64), ids[endpos].astype(np.int64)))
            else:
                ends_k.append((np.zeros(0, np.int64), np.zeros(0, np.int64)))
        nj = cap // jb
        distJ = np.ascontiguousarray(
            distT.reshape(cfg.n_sub, n_dist + 1, nj, jb).transpose(2, 1, 0, 3))
        af2J = np.ascontiguousarray(
            af2.reshape(2, 2 * ne1, nj, jb).transpose(2, 1, 0, 3))
        maskJ = np.ascontiguousarray(
            maskx.astype(np.dtype("float8_e4m3fn")).reshape(128, nj, jb)
            .transpose(1, 0, 2))
        in_maps.append(dict(
            distJ=distJ,
            af2J=af2J,
            maskJ=maskJ,
            a_fT_own=np.ascontiguousarray(af_aug[A0:A0 + cfg.apc].T),
            a_f_own=np.ascontiguousarray(af[A0:A0 + cfg.apc].T.astype(np.float32)),
            Wdf64=Wdf64.astype(bfloat16), Wcf2g=Wcf2g.astype(bfloat16),
            Wfc2g=Wfc2g.astype(bfloat16), bdf=bdf_col,
        ))
        post_data.append(ends_k)
    return in_maps, post_data, cap


def host_post(results, post_data, cfg):
    out = np.empty((cfg.n_atoms, cfg.n_emb), np.float32)
    for cid in range(cfg.n_cores):
        r = results[cid]
        agg = np.asarray(r["fin"]).astype(np.float32).T.copy()  # [apc, n_emb]
        sc = np.asarray(r["scanout"])  # bf16 [128, cap]
        for k in range(cfg.n_sub):
            endpos, atoms = post_data[cid][k]
            if len(endpos):
                vals = sc[32 * k:32 * k + cfg.n_emb][:, endpos].astype(np.float32)
                np.add.at(agg, atoms, vals.T)
        out[cid * cfg.apc:(cid + 1) * cfg.apc] = agg
    return out


_CACHE = {}


def kernel(**inputs):
    cfg = DEFAULT_CFG
    in_maps, post_data, cap = host_prep(inputs, cfg)
    if cap not in _CACHE:
        _CACHE[cap] = build_program(cfg, cap)
    nc = _CACHE[cap]
    res = run_bass_kernel_spmd(nc, in_maps, core_ids=list(range(cfg.n_cores)))
    return host_post(res.results, post_data, cfg)
